# revision 48
# baseline (speedup 1.0000x reference)
"""Bone_Direction_GCN fused kernel for 8 Trainium2 NeuronCores.

Data-parallel over batch: each core processes 2048 of 16384 batches.
Graph mixing is block-diagonal over groups of G=7 batches (119 rows).

v6: x is loaded ONCE per core, channel-major only (xT), halving input
DMA vs v4. Per 476-row macro tile (4 sub-tiles of 119 rows):

  PE : h = x W2t (rows-major, 4x64 cols)     z = x W1 (rows-major, 4x128)
       psH += ones x b2row (rank-1 b2 bias matmul, K=1)
       gT = A'-mix of LRh, two matmuls with 2 sub-tiles stacked in M
            (full partitions, half the cols); psG shares psH's PSUM bank
       psO = W4 applied to gT via zero-stacked stationaries [W4.T;0] /
             [0;W4.T] (HW forbids base-partition switches inside one
             PSUM accumulation group)
           += (M z)^T via z_ext stationary whose 2 extra K-rows add
              b1 (x ones-row) and b4 (x A-colsum row)
  Act: native Lrelu psH -> LRh (HW path; "max" mode for CoreSim
       numerics), psZ -> z_ext evac
  DVE: psG -> g_sb evac, psO -> outsb evac (bf16 cast)
  SP : input DMA (pairs of macro tiles) + consts; Pool: output DMA

PSUM: psHG 2 banks + psZ 2 + psO 3 + psT (dedicated tail bank) = 8.
Matmul emission is software-pipelined 3 tiles deep so PE never waits on
the Act/DVE evacuations of the same tile; the 68-row tail rides
mid-stream in its own PSUM bank. Output is the residual-free delta,
channel-major per tile; the host adds f32 x back.
"""

import sys

sys.path.insert(0, "/opt/trn_rl_repo")

import numpy as np
import ml_dtypes

B, J, E = 16384, 17, 32
CIN, COUT = 128, 128
MID = COUT // 2
PROP = 0.5
SLOPE = 0.01

N_CORES = 8
BC = B // N_CORES          # batches per core (2048)
ROWS = BC * J              # rows per core (34816)
G = 7                      # batches per sub-tile
R = G * J                  # rows per sub-tile (119)
S = 4                      # sub-tiles per macro-tile
RM = S * R                 # rows per macro-tile (476)
NM = 73                    # macro tiles per core (73*476 = 34748)
GT = BC - NM * S * G       # tail batches (4)
RT = GT * J                # tail rows (68)
SR = S * R                 # 476
SC = S * CIN               # 512
SM = S * MID               # 256
PW = 2 * SC                # psOp pair width, bank-aligned halves (1024)
NPAIR = NM // 2            # 36 (tile 72 is the odd one out)
TAIL_R = 40                # round where the 68-row tail is emitted
WARMUP_MM = 16             # PE p-state ramp matmuls before real work

assert NM * RM + RT == ROWS

_CACHE = {}


def _gcn_matrix(edge_index: np.ndarray, edge_weight: np.ndarray) -> np.ndarray:
    """Dense normalized GCN operator M with out[i] = sum_j M[i, j] * x[j]."""
    row = edge_index[0].astype(np.int64)
    col = edge_index[1].astype(np.int64)
    loop = np.arange(J, dtype=np.int64)
    row_f = np.concatenate([row, loop])
    col_f = np.concatenate([col, loop])
    w_f = np.concatenate([edge_weight.astype(np.float32), np.ones(J, np.float32)])
    deg = np.zeros(J, np.float32)
    np.add.at(deg, col_f, w_f)
    safe = np.where(deg > 0, deg, 1.0).astype(np.float32)
    dis = np.where(deg > 0, 1.0 / np.sqrt(safe), 0.0).astype(np.float32)
    norm = dis[row_f] * w_f * dis[col_f]
    M = np.zeros((J, J), np.float32)
    np.add.at(M, (col_f, row_f), norm)
    return M


def _block_diag(block: np.ndarray, n: int) -> np.ndarray:
    j = block.shape[0]
    out = np.zeros((n * j, n * j), block.dtype)
    for g in range(n):
        out[g * j:(g + 1) * j, g * j:(g + 1) * j] = block
    return out


def _mix_consts(M: np.ndarray, adj: np.ndarray, g: int):
    """mixMze [g*17+2, g*17]: rows 0:r = blockdiag(M.T) (so lhsT=z gives
    (Mz)^T), row r = ones (pairs z_ext b1 row), row r+1 = PROP*colsum(adj)
    tiled (pairs z_ext b4 row). mixA [g*17, g*17] = PROP*blockdiag(adj)
    (moving operand of the gT mix)."""
    r = g * J
    mixMze = np.zeros((r + 2, r), np.float32)
    mixMze[0:r] = _block_diag(M.T, g)
    mixMze[r] = 1.0
    mixMze[r + 1] = np.tile(PROP * adj.sum(axis=0), g)
    mixA = PROP * _block_diag(adj, g)
    return mixMze, mixA


def _build_bass(leaky_mode: str = "lrelu", warmup: int = WARMUP_MM,
                no_tail: bool = False, out_on_sp: bool = False,
                presets_on_sp: bool = False, stages: int = 3, **_ignored):
    import concourse.bacc as bacc
    import concourse.mybir as mybir
    import concourse.tile as tile
    from contextlib import ExitStack

    # leaky_mode="lrelu": native Lrelu on Act (HW path; CoreSim cannot
    # execute it numerically). "max": Act Identity(a*x) + DVE max (sim
    # numerics path; HW-legal too but one extra DVE op per tile).

    f32 = mybir.dt.float32
    bf16 = mybir.dt.bfloat16

    nc = bacc.Bacc("TRN2", target_bir_lowering=False, debug=False)

    xT_d = nc.dram_tensor("xT", [NM * CIN, SR], bf16, kind="ExternalInput").ap()
    xtT_d = nc.dram_tensor("xtT", [CIN, RT], bf16, kind="ExternalInput").ap()
    w21_d = nc.dram_tensor("w21", [CIN, MID + COUT], bf16, kind="ExternalInput").ap()
    mixA_d = nc.dram_tensor("mixA", [R, R], bf16, kind="ExternalInput").ap()
    mixMze_d = nc.dram_tensor("mixMze", [R + 2, R], bf16, kind="ExternalInput").ap()
    # zero-stacked W4 stationaries: w4e = [W4.T; 0], w4o = [0; W4.T].
    # Both read the full 128-partition g tile at base 0 -- matmuls within
    # one PSUM accumulation group must not switch base partition on HW.
    w4e_d = nc.dram_tensor("w4e", [2 * MID, COUT], bf16, kind="ExternalInput").ap()
    w4o_d = nc.dram_tensor("w4o", [2 * MID, COUT], bf16, kind="ExternalInput").ap()
    ones_d = nc.dram_tensor("onescol", [1, R], bf16, kind="ExternalInput").ap()
    b2row4_d = nc.dram_tensor("b2row4", [1, SM], bf16, kind="ExternalInput").ap()
    b14row_d = nc.dram_tensor("b14row", [2, SC], bf16, kind="ExternalInput").ap()
    mixAt_d = nc.dram_tensor("mixAt", [RT, RT], bf16, kind="ExternalInput").ap()
    mixMzte_d = nc.dram_tensor("mixMzte", [RT + 2, RT], bf16, kind="ExternalInput").ap()
    om_d = nc.dram_tensor("out", [NM * CIN, SR], bf16, kind="ExternalOutput").ap()
    ot_d = nc.dram_tensor("outt", [CIN, RT], bf16, kind="ExternalOutput").ap()

    with ExitStack() as ctx:
        tc = ctx.enter_context(tile.TileContext(nc))

        # input stream pool first: the lead-in pair load is emitted before
        # the const loads so SP/DMA start on the critical path immediately
        xin_pool = ctx.enter_context(tc.tile_pool(name="xin", bufs=5))
        st = {}

        def dma_in(p):
            """Load input pair p (tiles 2p, 2p+1); tile 72 loads single."""
            if p == NPAIR:
                xin = xin_pool.tile([CIN, SR], bf16, tag="xin_odd")
                nc.sync.dma_start(
                    out=xin[:], in_=xT_d[(NM - 1) * CIN:NM * CIN, :])
                st[("xin", NM - 1)] = xin[:]
            else:
                xin = xin_pool.tile([CIN, 2 * SR], bf16, tag="xin")
                nc.sync.dma_start(
                    out=xin[:].rearrange("p (t c) -> p t c", c=SR),
                    in_=xT_d[2 * p * CIN:(2 * p + 2) * CIN, :].rearrange(
                        "(t p) c -> p t c", p=CIN),
                )
                st[("xin", 2 * p)] = xin[:, 0:SR]
                st[("xin", 2 * p + 1)] = xin[:, SR:2 * SR]

        # lead-in: pair 0 loads on the idle Act HWDGE queue so its
        # transfer runs in parallel with the critical consts on SP
        import concourse.mybir as _mb
        const = ctx.enter_context(tc.tile_pool(name="const", bufs=1))
        SP = _mb.EngineType.SP
        POOL = _mb.EngineType.Pool
        xin0 = xin_pool.tile([CIN, 2 * SR], bf16, tag="xin")
        nc.scalar.dma_start(
            out=xin0[:].rearrange("p (t c) -> p t c", c=SR),
            in_=xT_d[0:2 * CIN, :].rearrange("(t p) c -> p t c", p=CIN),
        )
        st[("xin", 0)] = xin0[:, 0:SR]
        st[("xin", 1)] = xin0[:, SR:2 * SR]
        # critical-path consts ride SP between the two lead-in pair loads;
        # the rest go SWDGE on Pool so the Act queue stays clear
        w21_sb = const.tile_from(w21_d, forced_dma_engine=SP)
        mixA_sb = const.tile_from(mixA_d, forced_dma_engine=SP)
        mixMze_sb = const.tile_from(mixMze_d, forced_dma_engine=SP)
        w4e_sb = const.tile_from(w4e_d, forced_dma_engine=SP)
        w4o_sb = const.tile_from(w4o_d, forced_dma_engine=SP)
        ones_sb = const.tile_from(ones_d, forced_dma_engine=SP)
        b2row4_sb = const.tile_from(b2row4_d, forced_dma_engine=SP)
        mixAt_sb = const.tile_from(mixAt_d, forced_dma_engine=POOL)
        mixMzte_sb = const.tile_from(mixMzte_d, forced_dma_engine=POOL)
        xtT_sb = const.tile_from(xtT_d, forced_dma_engine=POOL)

        dma_in(1)

        # z_ext slots: rows 0:R = z (written per tile), rows R:R+2 = b1/b4
        # rows (preset once; pair with mixMze's ones/colsum rows)
        zext_pool = ctx.enter_context(tc.tile_pool(name="zext", bufs=4))
        zext_tiles = []
        for i in range(4):
            t = zext_pool.tile([R + 2, SC], bf16, tag=f"zext{i}")
            (nc.sync if presets_on_sp else nc.gpsimd).dma_start(
                out=t[R:R + 2, :], in_=b14row_d)
            zext_tiles.append(t)
        ztext_pool = ctx.enter_context(tc.tile_pool(name="ztext", bufs=1))
        ztext = ztext_pool.tile([RT + 2, CIN], bf16)
        (nc.sync if presets_on_sp else nc.gpsimd).dma_start(
            out=ztext[RT:RT + 2, :], in_=b14row_d[:, 0:CIN])

        lrh_pool = ctx.enter_context(tc.tile_pool(name="lrh", bufs=4))
        g_pool = ctx.enter_context(tc.tile_pool(name="gsb", bufs=4))
        out_pool = ctx.enter_context(tc.tile_pool(name="osb", bufs=4))

        # PSUM: psHG (h cols 0:256 + gT cols 256:494 share one bank) x2,
        # psZ x2, psO x3, psT (all tail regions in one dedicated bank) = 8
        psHG_pool = ctx.enter_context(tc.tile_pool(name="psHG", bufs=2, space="PSUM"))
        psZ_pool = ctx.enter_context(tc.tile_pool(name="psZ", bufs=2, space="PSUM"))
        psO_pool = ctx.enter_context(tc.tile_pool(name="psO", bufs=3, space="PSUM"))
        psT_pool = ctx.enter_context(tc.tile_pool(name="psT", bufs=1, space="PSUM"))

        # PE p-state warmup on a memset tile: no DMA dependency, so the
        # clock ramps while the first input pair loads (rides a psOp slot)
        if warmup:
            warm_pool = ctx.enter_context(tc.tile_pool(name="wrm", bufs=1))
            warm_sb = warm_pool.tile([CIN, CIN], bf16)
            nc.vector.memset(warm_sb[:], 1.0)
            # preload the activation table off the critical path
            warm_act = warm_pool.tile([CIN, 16], bf16, tag="wact")
            if leaky_mode == "lrelu":
                nc.scalar.activation(
                    warm_act[:], warm_sb[:, 0:16],
                    func=mybir.ActivationFunctionType.Lrelu,
                    scale=1.0, alpha=SLOPE,
                )
            wps = psO_pool.tile([CIN, SC], f32, tag="psO")
            for _ in range(warmup):
                nc.tensor.matmul(
                    wps[:, 0:CIN], lhsT=warm_sb[:], rhs=warm_sb[:],
                    start=True, stop=True,
                )

        def s1(m, xv):
            """mmH/mmZ/b2add for tile m; xv = [CIN, SR] channel-major view."""
            psHG = psHG_pool.tile([CIN, SC], f32, tag="psHG")
            psZ = psZ_pool.tile([R, SC], f32, tag="psZ")
            for s in range(S):
                chunk = xv[:, s * R:(s + 1) * R]
                nc.tensor.matmul(
                    psHG[0:R, s * MID:(s + 1) * MID], lhsT=chunk,
                    rhs=w21_sb[:, 0:MID],
                    start=(s == 0), stop=False, skip_group_check=True,
                )
                nc.tensor.matmul(
                    psZ[:, s * CIN:(s + 1) * CIN], lhsT=chunk,
                    rhs=w21_sb[:, MID:MID + CIN],
                    start=(s == 0), stop=(s == S - 1), skip_group_check=True,
                )
            nc.tensor.matmul(
                psHG[0:R, 0:SM], lhsT=ones_sb[:], rhs=b2row4_sb[:],
                start=False, stop=False, skip_group_check=True,
            )
            st[("psHG", m)] = psHG
            st[("psZ", m)] = psZ

        def leaky(lrh_ap, psh_ap, tag):
            """lrh = LeakyReLU(psh). lrelu: one Act op. max: Act
            Identity(a*x) then DVE max(psh, a*x) (one PSUM input each)."""
            if leaky_mode == "lrelu":
                nc.scalar.activation(
                    lrh_ap, psh_ap,
                    func=mybir.ActivationFunctionType.Lrelu,
                    scale=1.0, alpha=SLOPE,
                )
            else:
                a = lk_pool.tile(list(lrh_ap.shape), bf16, tag=f"lk_{tag}")
                nc.scalar.activation(
                    a[:], psh_ap,
                    func=mybir.ActivationFunctionType.Identity, scale=SLOPE,
                )
                nc.vector.scalar_tensor_tensor(
                    lrh_ap, psh_ap, 1.0, a[:],
                    op0=mybir.AluOpType.mult, op1=mybir.AluOpType.max,
                )

        lk_pool = ctx.enter_context(tc.tile_pool(name="lk", bufs=2))

        def ev1(m):
            """lrelu + z evac (both Act) for tile m."""
            psHG = st[("psHG", m)]
            psZ = st.pop(("psZ", m))
            lrh = lrh_pool.tile([R, SM], bf16, tag="lrh")
            leaky(lrh[:], psHG[0:R, 0:SM], "m")
            zext = zext_tiles[m % 4]
            nc.scalar.copy(zext[0:R, :], psZ[:])
            st[("lrh", m)] = lrh
            st[("zext", m)] = zext

        def s2(m):
            """gT mix for tile m: 2 matmuls, 2 sub-tiles stacked in M,
            into the g region (cols 256:494) of tile m's psHG bank."""
            lrh = st.pop(("lrh", m))
            psHG = st.pop(("psHG", m))
            # h=0 start=True re-marks the bank (safe: lrelu has already
            # consumed the H region) so all 128 partitions are covered
            for h in range(2):
                nc.tensor.matmul(
                    psHG[:, SM + h * R:SM + (h + 1) * R],
                    lhsT=lrh[:, h * CIN:(h + 1) * CIN], rhs=mixA_sb[:],
                    start=(h == 0), stop=(h == 1), skip_group_check=True,
                )
            st[("psG", m)] = psHG

        def ev2(m):
            """g evac (DVE) for tile m."""
            psHG = st.pop(("psG", m))
            gsb = g_pool.tile([2 * MID, 2 * R], bf16, tag="gsb")
            nc.vector.tensor_copy(gsb[:], psHG[:, SM:SM + 2 * R])
            st[("gsb", m)] = gsb

        def s3(m):
            """W4 apply + (Mz)^T + biases into psO for tile m."""
            gsb = st.pop(("gsb", m))
            zext = st.pop(("zext", m))
            psO = psO_pool.tile([CIN, SC], f32, tag="psO")
            for s in range(S):
                h = s % 2
                nc.tensor.matmul(
                    psO[:, s * R:(s + 1) * R],
                    lhsT=w4t_sb[h * MID:(h + 1) * MID, :],
                    rhs=gsb[h * MID:(h + 1) * MID,
                            (s // 2) * R:(s // 2) * R + R],
                    start=(s == 0), stop=False, skip_group_check=True,
                )
            for s in range(S):
                nc.tensor.matmul(
                    psO[:, s * R:(s + 1) * R],
                    lhsT=zext[:, s * CIN:(s + 1) * CIN], rhs=mixMze_sb[:],
                    start=False, stop=(s == S - 1), skip_group_check=True,
                )
            st[("psO", m)] = psO

        def ev3(m):
            """psO -> outsb half bf16 (DVE) for tile m."""
            psO = st.pop(("psO", m))
            ov = st.pop(("ov", m))
            nc.vector.tensor_copy(ov, psO[:, 0:SR])

        def alloc_out(p):
            if p == NPAIR:
                od = out_pool.tile([CIN, SR], bf16, tag="osb_odd")
                st[("ov", NM - 1)] = od[:]
            else:
                od = out_pool.tile([CIN, 2 * SR], bf16, tag="osb")
                st[("ov", 2 * p)] = od[:, 0:SR]
                st[("ov", 2 * p + 1)] = od[:, SR:2 * SR]
            st[("od", p)] = od

        def dma_out(p):
            od = st.pop(("od", p))
            # final output DMAs ride SP (idle at the end) to cut drain
            eng = nc.sync if (out_on_sp or p >= NPAIR - 1) else nc.gpsimd
            if p == NPAIR:
                eng.dma_start(
                    out=om_d[(NM - 1) * CIN:NM * CIN, :], in_=od[:])
            else:
                eng.dma_start(
                    out=om_d[2 * p * CIN:(2 * p + 2) * CIN, :].rearrange(
                        "(t p) c -> p t c", p=CIN),
                    in_=od[:].rearrange("p (t c) -> p t c", c=SR),
                )

        # ---- tail (68 rows): all PSUM regions live in the dedicated
        # psT bank so the main tile rings are never disturbed ----
        def s1_t():
            psT = psT_pool.tile([CIN, SC], f32)
            nc.tensor.matmul(psT[0:RT, 0:MID], lhsT=xtT_sb[:],
                             rhs=w21_sb[:, 0:MID],
                             start=True, stop=False, skip_group_check=True)
            nc.tensor.matmul(psT[0:RT, 200:200 + CIN], lhsT=xtT_sb[:],
                             rhs=w21_sb[:, MID:MID + CIN],
                             start=False, stop=False, skip_group_check=True)
            nc.tensor.matmul(psT[0:RT, 0:MID], lhsT=ones_sb[:, 0:RT],
                             rhs=b2row4_sb[:, 0:MID],
                             start=False, stop=False, skip_group_check=True)
            st["psT"] = psT

        def ev1_t():
            psT = st["psT"]
            lrh = lrh_pool.tile([RT, MID], bf16, tag="lrht")
            leaky(lrh[:], psT[0:RT, 0:MID], "t")
            nc.vector.tensor_copy(ztext[0:RT, :], psT[0:RT, 200:200 + CIN])
            st["lrht"] = lrh

        def s2_t():
            lrh = st.pop("lrht")
            psT = st["psT"]
            nc.tensor.matmul(psT[0:MID, MID:MID + RT], lhsT=lrh[:],
                             rhs=mixAt_sb[:],
                             start=False, stop=False, skip_group_check=True)

        def ev2_t():
            psT = st["psT"]
            gsb = g_pool.tile([MID, RT], bf16, tag="gsbt")
            nc.vector.tensor_copy(gsb[:], psT[0:MID, MID:MID + RT])
            st["gsbt"] = gsb

        def s3_t():
            gsb = st.pop("gsbt")
            psT = st["psT"]
            psO = psT[:, 132:132 + RT]
            nc.tensor.matmul(psO, lhsT=w4e_sb[0:MID, :], rhs=gsb[:],
                             start=True, stop=False, skip_group_check=True)
            nc.tensor.matmul(psO, lhsT=ztext[:], rhs=mixMzte_sb[:],
                             start=False, stop=True, skip_group_check=True)

        def ev3_t():
            psT = st.pop("psT")
            osb = out_pool.tile([CIN, RT], bf16, tag="osbt")
            nc.vector.tensor_copy(osb[:], psT[:, 132:132 + RT])
            st["osbt"] = osb

        def dma_out_t():
            nc.sync.dma_start(out=ot_d, in_=st.pop("osbt")[:])

        # ---- software-pipelined emission: rounds m = 0..NM+2 ----
        # round m: S1(m) | S2(m-1) | S3(m-2); the 68-row tail rides
        # mid-stream at round TAIL_R so it overlaps instead of draining.
        alloc_out(0)
        for m in range(NM + 3):
            if m < NM and m % 2 == 0 and m // 2 + 2 <= NPAIR:
                dma_in(m // 2 + 2)
            if m % 2 == 1 and (m + 1) // 2 <= NPAIR:
                alloc_out((m + 1) // 2)
            if m < NM:
                s1(m, st.pop(("xin", m)))
                ev1(m)
            if 0 <= m - 1 < NM and stages >= 2:
                s2(m - 1)
                ev2(m - 1)
            # tail stage 1 emits only after the s2/ev2 of the previous
            # psHG slot user, so slot lifecycles stay in emission order
            if m == TAIL_R and not no_tail:
                s1_t()
                ev1_t()
            if m - 1 == TAIL_R and not no_tail:
                s2_t()
                ev2_t()
            if 0 <= m - 2 < NM and stages >= 3:
                s3(m - 2)
                ev3(m - 2)
            elif 0 <= m - 2 < NM and stages < 3:
                # stub: dump zext into the out tile so outputs are defined
                zext = st.pop(("zext", m - 2))
                if stages >= 2:
                    st.pop(("gsb", m - 2))
                else:
                    st.pop(("lrh", m - 2))
                ov = st.pop(("ov", m - 2))
                nc.vector.tensor_copy(ov[0:R, :], zext[0:R, 0:SR])
            if m - 2 == TAIL_R and not no_tail:
                s3_t()
                ev3_t()
                dma_out_t()
            # pair p = (m-3)//2 fully evacuated at end of odd round m = 2p+3
            if m >= 3 and m % 2 == 1 and (m - 3) // 2 <= NPAIR:
                dma_out((m - 3) // 2)

    nc.compile()
    return nc


def _host_consts(inputs):
    bf = ml_dtypes.bfloat16
    M = _gcn_matrix(np.asarray(inputs["edge_index"]), np.asarray(inputs["edge_weight"]))
    adj = np.asarray(inputs["adj"], np.float32)
    mixMze, mixA = _mix_consts(M, adj, G)
    mixMzte, mixAt = _mix_consts(M, adj, GT)
    W1 = np.asarray(inputs["W1"], np.float32)
    W2 = np.asarray(inputs["W2"], np.float32)
    W4 = np.asarray(inputs["W4"], np.float32)
    b1 = np.asarray(inputs["b1"], np.float32)
    b2 = np.asarray(inputs["b2"], np.float32)
    b4 = np.asarray(inputs["b4"], np.float32)
    w21 = np.concatenate([W2.T, W1], axis=1)  # [128, 192]
    b14 = np.stack([np.tile(b1, S), np.tile(b4, S)], axis=0)  # [2, 512]
    return {
        "w21": np.ascontiguousarray(w21).astype(bf),
        "mixA": mixA.astype(bf),
        "mixMze": mixMze.astype(bf),
        "w4e": np.ascontiguousarray(np.concatenate(
            [W4.T, np.zeros((MID, COUT), np.float32)], axis=0)).astype(bf),
        "w4o": np.ascontiguousarray(np.concatenate(
            [np.zeros((MID, COUT), np.float32), W4.T], axis=0)).astype(bf),
        "onescol": np.ones((1, R), np.float32).astype(bf),
        "b2row4": np.tile(b2, S)[None, :].astype(bf),
        "b14row": b14.astype(bf),
        "mixAt": mixAt.astype(bf),
        "mixMzte": mixMzte.astype(bf),
    }


def _core_x(vector: np.ndarray, c: int) -> dict:
    """Per-core input, bf16, channel-major per macro tile."""
    bf = ml_dtypes.bfloat16
    x2 = vector[c * BC:(c + 1) * BC].reshape(ROWS, CIN)
    main = x2[:NM * RM].reshape(NM, S, R, CIN)
    xT = main.transpose(0, 3, 1, 2).reshape(NM * CIN, SR)
    tail = x2[NM * RM:]
    return {
        "xT": np.ascontiguousarray(xT).astype(bf),
        "xtT": np.ascontiguousarray(tail.T).astype(bf),
    }


def _assemble_out(
    vector: np.ndarray, c: int, out_m: np.ndarray, out_t: np.ndarray
) -> np.ndarray:
    """Invert the channel-major permutation and add the f32 residual."""
    dm = np.asarray(out_m, np.float32)
    main = dm.reshape(NM, CIN, S, R).transpose(0, 2, 3, 1).reshape(NM * RM, CIN)
    dt = np.asarray(out_t, np.float32).T
    delta = np.concatenate([main, dt], axis=0)
    x2 = vector[c * BC:(c + 1) * BC].reshape(ROWS, CIN)
    return (x2 + delta).reshape(BC, J, CIN)


def kernel(**inputs) -> np.ndarray:
    from concourse.bass_utils import run_bass_kernel_spmd

    if "nc" not in _CACHE:
        _CACHE["nc"] = _build_bass()
    nc = _CACHE["nc"]

    consts = _host_consts(inputs)
    vector = np.ascontiguousarray(np.asarray(inputs["vector"], np.float32))
    in_maps = []
    for c in range(N_CORES):
        m = dict(consts)
        m.update(_core_x(vector, c))
        in_maps.append(m)

    res = run_bass_kernel_spmd(nc, in_maps, core_ids=list(range(N_CORES)))
    outs = [
        _assemble_out(vector, c, res.results[c]["out"], res.results[c]["outt"])
        for c in range(N_CORES)
    ]
    return np.concatenate(outs, axis=0)
